# revision 1
# baseline (speedup 1.0000x reference)
"""H2GCN neighborhood aggregation on 8 Trainium2 NeuronCores.

Computes concat([adj_t @ x, adj_t2 @ x], axis=1) for
adj_t/adj_t2: [8192, 8192] f32, x: [8192, 256] f32.

Sharding: row-shard adj_t/adj_t2 (1024 rows per core), replicate x,
each core produces its [1024, 512] slice of the output.

fp8dr mode (default): adjacency is centered (a - 0.5) and quantized to
fp8 e4m3 on host, x quantized to e4m3, and the rank-1 term
0.5 * colsum(x) is carried exactly in f32 and added after accumulation.
Matmuls run in MatmulPerfMode.DoubleRow (2 k-subtiles per instruction,
2x PE throughput) and adjacency HBM traffic is halved vs bf16.
Measured rel err vs f64 reference: 1.4e-2 (gate 2e-2).

Per-core dataflow:
  - host pre-transposes the adjacency slice to [8192 k, 1024 m] e4m3 so
    stationary operands load with plain contiguous DMA (no PE/DMA
    transposes).
  - 32 chunks per matrix of [128p, 2, 1024m] (256 KB, k-pair stripes),
    each feeding 8 DoubleRow matmuls (one per 128-row output block)
    that accumulate over kk into 8 PSUM banks [128, 256] f32.
  - DVE adds the f32 colsum correction during PSUM -> SBUF copy.
"""

import numpy as np

N = 8192
D = 256
CORES = 8
P = 128
M_LOC = N // CORES  # 1024 rows of each adjacency matrix per core
MB = M_LOC // P  # 8 output row-blocks per core
KB = N // P  # 64 contraction blocks
KK = N // (2 * P)  # 32 k-pair blocks (DoubleRow consumes 256 rows/step)
GRP = 8  # k-blocks per transpose/copy group (one PSUM bank), bf16t mode
N_GRP = KB // GRP  # 8

MODE = "fp8drb"  # "fp8drb"/"fp8dr" (e4m3 DoubleRow) or "bf16t"

TUNE = dict(
    a_bufs=12,  # fp8 chunk slots in flight
    xg=4,  # kk-blocks per x-load DMA
    chunk_kk=1,  # kk-pairs per adjacency chunk DMA
    out_bf16=True,  # store out as bf16, host upcasts
    diag="none",  # "nodma" (tiny chunk DMAs) / "nomm" (single matmul per acc)
    hwloop=True,  # use tc.For_i for the repeat loop (timing NEFFs only)
    il=True,  # host pair-interleaves adjacency rows (2KB contiguous/partition)
)

_cache = {}


def _build_fp8drb(repeat=1):
    """x-stationary orientation: out^T[d, m] accumulates in PSUM while the
    adjacency chunk streams as the 1024-wide moving operand (512 k-pairs).
    Stationary x blocks are reused across 2 moving streams, so LD_WEIGHTS
    is amortized 4x vs the a-stationary orientation. The colsum correction
    is a per-partition bias add; host un-transposes the [512, 1024] out.
    Output is stored bf16 (host upcasts); x loads stream on the gpsimd
    DMA queue interleaved with the first matrix's chunks."""
    import concourse.bacc as bacc
    import concourse.tile as tile
    import concourse.mybir as mybir

    F32 = mybir.dt.float32
    FP8 = mybir.dt.float8e4
    ODT = mybir.dt.bfloat16 if TUNE["out_bf16"] else F32
    DR = mybir.MatmulPerfMode.DoubleRow

    nc = bacc.Bacc(
        "TRN2",
        target_bir_lowering=False,
        debug=False,
        enable_asserts=False,
        num_devices=CORES,
    )
    at_ap = nc.dram_tensor("at", [N, M_LOC], FP8, kind="ExternalInput").ap()
    a2t_ap = nc.dram_tensor("a2t", [N, M_LOC], FP8, kind="ExternalInput").ap()
    x_ap = nc.dram_tensor("x", [N, D], FP8, kind="ExternalInput").ap()
    c_ap = nc.dram_tensor("c", [P, 2], F32, kind="ExternalInput").ap()
    # out is transposed: [2D, M_LOC]; host transposes back (and upcasts)
    out_ap = nc.dram_tensor("out", [2 * D, M_LOC], ODT, kind="ExternalOutput").ap()

    XG = TUNE["xg"]
    CKK = TUNE["chunk_kk"]  # kk-pairs per adjacency chunk DMA
    DBLK = D // P  # 2 stationary d-blocks
    MC = M_LOC // 512  # 2 moving chunks of 512 columns
    with tile.TileContext(nc) as tc:
        with (
            tc.tile_pool(name="xp", bufs=1) as x_pool,
            tc.tile_pool(name="cp", bufs=1) as c_pool,
            tc.tile_pool(name="ap", bufs=TUNE["a_bufs"]) as a_pool,
            tc.tile_pool(name="op", bufs=2 * DBLK * MC) as o_pool,
            tc.tile_pool(name="pacc", bufs=2 * DBLK * MC, space="PSUM") as acc_pool,
        ):
            c_t = c_pool.tile([P, 2], F32)
            nc.gpsimd.dma_start(c_t[:], c_ap[:])

            x_t = x_pool.tile([P, KK, 2, D], FP8)
            x_re = x_ap.rearrange("(kk i p) d -> p kk i d", p=P, i=2)

            for _rep in range(repeat):
                for mat, src_ap in ((0, at_ap), (1, a2t_ap)):
                    accs = [
                        acc_pool.tile([P, 512], F32, tag="acc", name=f"acc{i}")
                        for i in range(DBLK * MC)
                    ]
                    src_re = src_ap.rearrange("(kk i p) m -> p kk i m", p=P, i=2)
                    for kk0 in range(0, KK, CKK):
                        if _rep == 0 and mat == 0 and kk0 % XG == 0:
                            # x chunk rides the gpsimd queue, just ahead of
                            # the adjacency chunks that need it
                            nc.gpsimd.dma_start(
                                x_t[:, kk0 : kk0 + XG], x_re[:, kk0 : kk0 + XG]
                            )
                        ch = a_pool.tile([P, CKK, 2, M_LOC], FP8, tag="achunk")
                        nc.sync.dma_start(ch[:], src_re[:, kk0 : kk0 + CKK])
                        for j in range(CKK):
                            kk = kk0 + j
                            for db in range(DBLK):
                                for mc in range(MC):
                                    nc.tensor.matmul(
                                        accs[db * MC + mc][:],
                                        x_t[:, kk, :, db * P : (db + 1) * P],
                                        ch[:, j, :, mc * 512 : (mc + 1) * 512],
                                        start=(kk == 0),
                                        stop=(kk == KK - 1),
                                        perf_mode=DR,
                                    )
                    for db in range(DBLK):
                        for mc in range(MC):
                            ot = o_pool.tile([P, 512], ODT, tag="outt")
                            if (db * MC + mc) % 2 == 0:
                                nc.vector.tensor_scalar_add(
                                    ot[:], accs[db * MC + mc][:], c_t[:, db : db + 1]
                                )
                            else:
                                nc.scalar.add(
                                    ot[:], accs[db * MC + mc][:], c_t[:, db : db + 1]
                                )
                            nc.scalar.dma_start(
                                out_ap[
                                    mat * D + db * P : mat * D + (db + 1) * P,
                                    mc * 512 : (mc + 1) * 512,
                                ],
                                ot[:],
                            )

    nc.compile()
    return nc


def _build_fp8dr(repeat=1):
    import concourse.bacc as bacc
    import concourse.tile as tile
    import concourse.mybir as mybir

    F32 = mybir.dt.float32
    FP8 = mybir.dt.float8e4
    DR = mybir.MatmulPerfMode.DoubleRow

    nc = bacc.Bacc(
        "TRN2",
        target_bir_lowering=False,
        debug=False,
        enable_asserts=False,
        num_devices=CORES,
    )
    at_ap = nc.dram_tensor("at", [N, M_LOC], FP8, kind="ExternalInput").ap()
    a2t_ap = nc.dram_tensor("a2t", [N, M_LOC], FP8, kind="ExternalInput").ap()
    x_ap = nc.dram_tensor("x", [N, D], FP8, kind="ExternalInput").ap()
    c_ap = nc.dram_tensor("c", [P, D], F32, kind="ExternalInput").ap()
    out_ap = nc.dram_tensor("out", [M_LOC, 2 * D], F32, kind="ExternalOutput").ap()

    XG = TUNE["xg"]
    with tile.TileContext(nc) as tc:
        with (
            tc.tile_pool(name="xp", bufs=1) as x_pool,
            tc.tile_pool(name="cp", bufs=1) as c_pool,
            tc.tile_pool(name="ap", bufs=TUNE["a_bufs"]) as a_pool,
            tc.tile_pool(name="op", bufs=MB) as o_pool,
            tc.tile_pool(name="pacc", bufs=MB, space="PSUM") as acc_pool,
        ):
            c_t = c_pool.tile([P, D], F32)
            nc.sync.dma_start(c_t[:], c_ap[:])

            x_t = x_pool.tile([P, KK, 2, D], FP8)
            x_re = x_ap.rearrange("(kk i p) d -> p kk i d", p=P, i=2)
            for g in range(KK // XG):
                nc.sync.dma_start(
                    x_t[:, g * XG : (g + 1) * XG], x_re[:, g * XG : (g + 1) * XG]
                )

            out_ts = [
                o_pool.tile([P, 2 * D], F32, tag="outt", name=f"outt{i}")
                for i in range(MB)
            ]
            for _rep in range(repeat):
                for mat, src_ap in ((0, at_ap), (1, a2t_ap)):
                    accs = [
                        acc_pool.tile([P, D], F32, tag="acc", name=f"acc{i}")
                        for i in range(MB)
                    ]
                    src_re = src_ap.rearrange("(kk i p) m -> p kk i m", p=P, i=2)
                    for kk in range(KK):
                        ch = a_pool.tile([P, 2, M_LOC], FP8, tag="achunk")
                        nc.sync.dma_start(ch[:], src_re[:, kk])
                        for mb in range(MB):
                            nc.tensor.matmul(
                                accs[mb][:],
                                ch[:, :, mb * P : (mb + 1) * P],
                                x_t[:, kk],
                                start=(kk == 0),
                                stop=(kk == KK - 1),
                                perf_mode=DR,
                            )
                    for mb in range(MB):
                        nc.vector.tensor_add(
                            out_ts[mb][:, mat * D : (mat + 1) * D],
                            accs[mb][:],
                            c_t[:],
                        )
                for mb in range(MB):
                    nc.sync.dma_start(out_ap[mb * P : (mb + 1) * P, :], out_ts[mb][:])

    nc.compile()
    return nc


def _build_bf16t(repeat=1):
    """Inputs pre-cast to bf16 on host (halves adjacency HBM traffic).
    A column-stripes [1024, 128] are loaded via the HW xbar DMA-transpose
    directly into matmul-ready [128k, 1024m] layout — no PE transposes, no
    PSUM round-trip. 8 PSUM banks hold one accumulator per output row-block."""
    import concourse.bacc as bacc
    import concourse.tile as tile
    import concourse.mybir as mybir

    F32 = mybir.dt.float32
    BF16 = mybir.dt.bfloat16

    nc = bacc.Bacc(
        "TRN2",
        target_bir_lowering=False,
        debug=False,
        enable_asserts=False,
        num_devices=CORES,
    )
    a_ap = nc.dram_tensor("a", [M_LOC, N], BF16, kind="ExternalInput").ap()
    a2_ap = nc.dram_tensor("a2", [M_LOC, N], BF16, kind="ExternalInput").ap()
    x_ap = nc.dram_tensor("x", [N, D], BF16, kind="ExternalInput").ap()
    out_ap = nc.dram_tensor("out", [M_LOC, 2 * D], F32, kind="ExternalOutput").ap()

    with tile.TileContext(nc) as tc:
        with (
            tc.tile_pool(name="xp", bufs=1) as x_pool,
            tc.tile_pool(name="stp", bufs=6) as st_pool,
            tc.tile_pool(name="op", bufs=MB) as o_pool,
            tc.tile_pool(name="pacc", bufs=MB, space="PSUM") as acc_pool,
        ):
            x_t = x_pool.tile([P, KB, D], BF16)
            x_re = x_ap.rearrange("(j p) d -> p j d", p=P)
            for g in range(N_GRP):
                nc.sync.dma_start(
                    x_t[:, g * GRP : (g + 1) * GRP, :],
                    x_re[:, g * GRP : (g + 1) * GRP, :],
                )

            out_ts = [
                o_pool.tile([P, 2 * D], F32, tag="outt", name=f"outt{i}")
                for i in range(MB)
            ]
            for _rep in range(repeat):
                for mat, src_ap in ((0, a_ap), (1, a2_ap)):
                    accs = [
                        acc_pool.tile([P, D], F32, tag="acc", name=f"acc{i}")
                        for i in range(MB)
                    ]
                    for k in range(KB):
                        st = st_pool.tile([P, M_LOC], BF16, tag="stripe")
                        nc.sync.dma_start_transpose(
                            st[:], src_ap[:, k * P : (k + 1) * P]
                        )
                        for mb in range(MB):
                            nc.tensor.matmul(
                                accs[mb][:],
                                st[:, mb * P : (mb + 1) * P],
                                x_t[:, k, :],
                                start=(k == 0),
                                stop=(k == KB - 1),
                            )
                    for mb in range(MB):
                        if mb % 2 == 0:
                            nc.vector.tensor_copy(
                                out_ts[mb][:, mat * D : (mat + 1) * D], accs[mb][:]
                            )
                        else:
                            nc.scalar.copy(
                                out_ts[mb][:, mat * D : (mat + 1) * D], accs[mb][:]
                            )
                for mb in range(MB):
                    nc.sync.dma_start(out_ap[mb * P : (mb + 1) * P, :], out_ts[mb][:])

    nc.compile()
    return nc


def _build_fp8drc(repeat=1):
    """Like fp8drb but each stationary x(kk, db) load feeds BOTH matrices'
    moving streams (4x 512-pair streams per LDWEIGHTS, 64 loads total).
    All 8 PSUM banks hold the two matrices' accumulators simultaneously."""
    import concourse.bacc as bacc
    import concourse.tile as tile
    import concourse.mybir as mybir

    F32 = mybir.dt.float32
    FP8 = mybir.dt.float8e4
    ODT = mybir.dt.bfloat16 if TUNE["out_bf16"] else F32
    DR = mybir.MatmulPerfMode.DoubleRow

    nc = bacc.Bacc(
        "TRN2",
        target_bir_lowering=False,
        debug=False,
        enable_asserts=False,
        num_devices=CORES,
    )
    at_ap = nc.dram_tensor("at", [N, M_LOC], FP8, kind="ExternalInput").ap()
    a2t_ap = nc.dram_tensor("a2t", [N, M_LOC], FP8, kind="ExternalInput").ap()
    x_ap = nc.dram_tensor("x", [N, D], FP8, kind="ExternalInput").ap()
    c_ap = nc.dram_tensor("c", [P, 2], F32, kind="ExternalInput").ap()
    out_ap = nc.dram_tensor("out", [2 * D, M_LOC], ODT, kind="ExternalOutput").ap()

    XG = TUNE["xg"]
    DBLK = D // P  # 2 stationary d-blocks
    MC = M_LOC // 512  # 2 moving chunks of 512 columns
    with tile.TileContext(nc) as tc:
        with (
            tc.tile_pool(name="xp", bufs=1) as x_pool,
            tc.tile_pool(name="cp", bufs=1) as c_pool,
            tc.tile_pool(name="ap", bufs=TUNE["a_bufs"]) as a_pool,
            tc.tile_pool(name="op", bufs=2 * DBLK * MC) as o_pool,
            tc.tile_pool(name="pacc", bufs=2 * DBLK * MC, space="PSUM") as acc_pool,
        ):
            c_t = c_pool.tile([P, 2], F32)
            nc.gpsimd.dma_start(c_t[:], c_ap[:])

            x_t = x_pool.tile([P, KK, 2, D], FP8)
            x_re = x_ap.rearrange("(kk i p) d -> p kk i d", p=P, i=2)

            if TUNE["il"]:
                # host stored rows as (kk, p, i): per partition the k-pair is
                # one contiguous 2048 B run
                at_re = at_ap.rearrange("(kk p i) m -> p kk i m", p=P, i=2)
                a2t_re = a2t_ap.rearrange("(kk p i) m -> p kk i m", p=P, i=2)
            else:
                at_re = at_ap.rearrange("(kk i p) m -> p kk i m", p=P, i=2)
                a2t_re = a2t_ap.rearrange("(kk i p) m -> p kk i m", p=P, i=2)

            hwloop = repeat > 1 and TUNE["hwloop"]
            if hwloop:
                # hardware loop: x fully loaded upfront, body emitted once
                for g in range(KK // XG):
                    nc.gpsimd.dma_start(
                        x_t[:, g * XG : (g + 1) * XG], x_re[:, g * XG : (g + 1) * XG]
                    )
                rep_iter = [0]
                loop_cm = tc.For_i(0, repeat)
                loop_cm.__enter__()
            else:
                rep_iter = list(range(repeat))
            for _rep in rep_iter:
                # acc index: mat*4 + db*2 + mc
                accs = [
                    acc_pool.tile([P, 512], F32, tag="acc", name=f"acc{i}")
                    for i in range(2 * DBLK * MC)
                ]
                diag = TUNE["diag"]
                for kk in range(KK):
                    if not hwloop and _rep == 0 and kk % XG == 0:
                        nc.gpsimd.dma_start(
                            x_t[:, kk : kk + XG], x_re[:, kk : kk + XG]
                        )
                    ch = a_pool.tile([P, 2, M_LOC], FP8, tag="achunk")
                    ch2 = a_pool.tile([P, 2, M_LOC], FP8, tag="achunk")
                    if diag == "nodma":
                        # 1/32 of the bytes: keeps the dep structure, removes
                        # the DMA load so PE-only time is visible
                        nc.sync.dma_start(ch[:, :, :32], at_re[:, kk, :, :32])
                        nc.scalar.dma_start(ch2[:, :, :32], a2t_re[:, kk, :, :32])
                    else:
                        # two HWDGE queues in parallel: at on sync, a2t on
                        # scalar (each ~8 MB/rep)
                        nc.sync.dma_start(ch[:], at_re[:, kk])
                        nc.scalar.dma_start(ch2[:], a2t_re[:, kk])
                    if diag == "nomm" and kk > 0:
                        continue
                    stop_kk = 0 if diag == "nomm" else KK - 1
                    for db in range(DBLK):
                        for mat, c_ch in ((0, ch), (1, ch2)):
                            for mc in range(MC):
                                nc.tensor.matmul(
                                    accs[mat * 4 + db * MC + mc][:],
                                    x_t[:, kk, :, db * P : (db + 1) * P],
                                    c_ch[:, :, mc * 512 : (mc + 1) * 512],
                                    start=(kk == 0),
                                    stop=(kk == stop_kk),
                                    perf_mode=DR,
                                )
                for mat in range(2):
                    for db in range(DBLK):
                        for mc in range(MC):
                            ot = o_pool.tile([P, 512], ODT, tag="outt")
                            if (db * MC + mc) % 2 == 0:
                                nc.vector.tensor_scalar_add(
                                    ot[:], accs[mat * 4 + db * MC + mc][:],
                                    c_t[:, db : db + 1],
                                )
                            else:
                                nc.scalar.add(
                                    ot[:], accs[mat * 4 + db * MC + mc][:],
                                    c_t[:, db : db + 1],
                                )
                            nc.scalar.dma_start(
                                out_ap[
                                    mat * D + db * P : mat * D + (db + 1) * P,
                                    mc * 512 : (mc + 1) * 512,
                                ],
                                ot[:],
                            )
            if hwloop:
                loop_cm.__exit__(None, None, None)

    nc.compile()
    return nc


def _build(mode, repeat=1):
    if mode == "fp8drc":
        return _build_fp8drc(repeat)
    if mode == "fp8drb":
        return _build_fp8drb(repeat)
    if mode == "fp8dr":
        return _build_fp8dr(repeat)
    if mode == "bf16t":
        return _build_bf16t(repeat)
    raise ValueError(f"unknown mode {mode}")


def _get_nc(mode, repeat=1):
    key = (mode, repeat, tuple(sorted(TUNE.items())))
    if key not in _cache:
        _cache[key] = _build(mode, repeat)
    return _cache[key]


def make_in_maps(x, adj_t, adj_t2, mode=MODE):
    import ml_dtypes

    x = np.ascontiguousarray(np.asarray(x, dtype=np.float32))
    adj_t = np.asarray(adj_t, dtype=np.float32)
    adj_t2 = np.asarray(adj_t2, dtype=np.float32)
    if mode in ("fp8dr", "fp8drb", "fp8drc"):
        e4 = ml_dtypes.float8_e4m3
        xq = x.astype(e4)
        c_row = (0.5 * x.sum(0, dtype=np.float64)).astype(np.float32)
        if mode in ("fp8drb", "fp8drc"):
            c = np.ascontiguousarray(c_row.reshape(2, P).T)  # [P, 2] d-blocks
        else:
            c = np.ascontiguousarray(np.broadcast_to(c_row, (P, D)))
        def prep(adj, sl):
            at = np.ascontiguousarray((adj[sl] - 0.5).astype(e4).T)  # [N, M_LOC]
            if mode == "fp8drc" and TUNE["il"]:
                # reorder k-rows (kk, i, p) -> (kk, p, i) so each partition's
                # DoubleRow pair is contiguous in DRAM
                at = np.ascontiguousarray(
                    at.reshape(KK, 2, P, M_LOC).swapaxes(1, 2).reshape(N, M_LOC)
                )
            return at

        maps = []
        for cid in range(CORES):
            sl = slice(cid * M_LOC, (cid + 1) * M_LOC)
            maps.append(
                {
                    "at": prep(adj_t, sl),
                    "a2t": prep(adj_t2, sl),
                    "x": xq,
                    "c": c,
                }
            )
        return maps
    bf = ml_dtypes.bfloat16
    xb = x.astype(bf)
    ab = adj_t.astype(bf)
    a2b = adj_t2.astype(bf)
    return [
        {
            "a": ab[c * M_LOC : (c + 1) * M_LOC],
            "a2": a2b[c * M_LOC : (c + 1) * M_LOC],
            "x": xb,
        }
        for c in range(CORES)
    ]


def gather_out(results, mode=MODE):
    if mode in ("fp8drb", "fp8drc"):
        return np.concatenate(
            [np.ascontiguousarray(r["out"].T).astype(np.float32) for r in results],
            axis=0,
        )
    return np.concatenate([r["out"] for r in results], axis=0)


def kernel(x, adj_t, adj_t2):
    from concourse.bass_utils import run_bass_kernel_spmd

    nc = _get_nc(MODE)
    in_maps = make_in_maps(x, adj_t, adj_t2, MODE)
    res = run_bass_kernel_spmd(nc, in_maps, core_ids=list(range(CORES)))
    return gather_out(res.results, MODE)

